# revision 26
# baseline (speedup 1.0000x reference)
"""Local self-attention (window=65) Trainium2 kernel, 8 NeuronCores.

Sharding: 4096 tokens (B*S flattened) split into 8 shards of 512 tokens.
Each core gets a halo'd, pre-transposed x slice plus replicated weights
(halo = 32 tokens each side, zero-padded at batch-sequence edges; zero x
tokens produce exactly-zero k/v since the qkv projection has no bias,
matching the reference's zero-padding semantics).

Per-core pipeline (Bass/Tile, bf16 matmuls with fp32 accumulation):
  1. qT/kT projections (feature-major) and v projection (token-major);
     a partition-shifted copy of v serves the unaligned second half of
     each 192-key strip.
  2. Per 128-query block x head: banded scores [128,192] on PE,
     exp on ACT straight out of PSUM, multiplicative band-mask fused
     with the row-sum in one DVE scalar_tensor_tensor, normalize, then
     transpose the weights via DMA-XBAR (no PE transposes) and run the
     two AV matmuls back into feature-major layout.
  3. Per-block output projection + bias, DMA out.
"""

import numpy as np
import ml_dtypes

import concourse.bass as bass
import concourse.mybir as mybir
import concourse.tile as tile
from concourse import bacc
from concourse.bass_utils import run_bass_kernel_spmd

F32 = mybir.dt.float32
BF16 = mybir.dt.bfloat16

# ---- problem constants (hardcoded) ----
B, S, DM = 2, 2048, 512
H, D, WIN = 8, 64, 65
PAD = WIN // 2              # 32
NCORES = 8
SHARD = B * S // NCORES     # 512 tokens per core
HALO = SHARD + 2 * PAD      # 576
NBLK = SHARD // 128         # 4 query blocks
KEYS = 128 + 2 * PAD        # 192 keys per block
NFT = DM // 128             # 4 feature tiles

DT_PROJ = BF16   # x / Wq / Wk / Wv sbuf dtype (projection matmuls)
DT_ATTN = BF16   # qT/kT/v/e/w/attnT/Wo sbuf dtype (attention + out-proj)


def _np_dt(dt):
    return {F32: np.float32, BF16: ml_dtypes.bfloat16}[dt]


def _build_program(stage="full"):
    nc = bacc.Bacc("TRN2", target_bir_lowering=False, debug=False)

    xT_d = nc.dram_tensor("xT", [DM, HALO], DT_PROJ, kind="ExternalInput")
    Wq_d = nc.dram_tensor("Wq", [DM, DM], DT_PROJ, kind="ExternalInput")
    Wk_d = nc.dram_tensor("Wk", [DM, DM], DT_PROJ, kind="ExternalInput")
    Wv_d = nc.dram_tensor("Wv", [DM, DM], DT_PROJ, kind="ExternalInput")
    Wo_d = nc.dram_tensor("Wo", [DM, DM], DT_ATTN, kind="ExternalInput")
    bias_d = nc.dram_tensor("bias", [DM], F32, kind="ExternalInput")
    mask_d = nc.dram_tensor("mask2", [128, KEYS], DT_ATTN, kind="ExternalInput")
    out_d = nc.dram_tensor("out", [SHARD, DM], F32, kind="ExternalOutput")

    Exp = mybir.ActivationFunctionType.Exp
    Copy = mybir.ActivationFunctionType.Copy
    Mult = mybir.AluOpType.mult

    cp_idx = [0]

    def copy_alt(out, in_):
        # alternate PSUM->SBUF copies between DVE and ACT to balance load
        if cp_idx[0] % 2 == 0:
            nc.vector.tensor_copy(out=out, in_=in_)
        else:
            nc.scalar.activation(out=out, in_=in_, func=Copy)
        cp_idx[0] += 1

    with tile.TileContext(nc) as tc:
        with (
            tc.tile_pool(name="consts", bufs=1) as cpool,
            tc.tile_pool(name="qkv", bufs=1) as qpool,
            tc.tile_pool(name="work", bufs=4) as wpool,
            tc.tile_pool(name="outp", bufs=2) as opool,
            tc.tile_pool(name="pp", bufs=2, space="PSUM") as pp,
            tc.tile_pool(name="ps", bufs=4, space="PSUM") as ps,
            tc.tile_pool(name="pa", bufs=2, space="PSUM") as pa,
        ):
            # ---- load constants ----
            xT_sb, Wq_sb, Wk_sb, Wv_sb, Wo_sb = [], [], [], [], []
            for kc in range(NFT):
                t = cpool.tile([128, HALO], DT_PROJ, tag=f"xT{kc}")
                nc.sync.dma_start(out=t[:], in_=xT_d[128 * kc:128 * (kc + 1), :])
                xT_sb.append(t)
            for nm, dram, lst, dt in (
                ("Wq", Wq_d, Wq_sb, DT_PROJ),
                ("Wk", Wk_d, Wk_sb, DT_PROJ),
                ("Wv", Wv_d, Wv_sb, DT_PROJ),
                ("Wo", Wo_d, Wo_sb, DT_ATTN),
            ):
                for kc in range(NFT):
                    t = cpool.tile([128, DM], dt, tag=f"{nm}{kc}")
                    nc.sync.dma_start(out=t[:], in_=dram[128 * kc:128 * (kc + 1), :])
                    lst.append(t)
            bias_sb = cpool.tile([128, DM], F32, tag="bias")
            bias_ap = bias_d[:]
            nc.gpsimd.dma_start(
                out=bias_sb[:],
                in_=bass.AP(tensor=bias_ap.tensor, offset=bias_ap.offset,
                            ap=[[0, 128]] + list(bias_ap.ap)),
            )
            mask_sb = cpool.tile([128, KEYS], DT_ATTN, tag="mask")
            nc.sync.dma_start(out=mask_sb[:], in_=mask_d[:, :])

            # ---- projections ----
            qT_sb, kT_sb, v_sb = [], [], []
            for ft in range(NFT):
                csl = slice(ft * 128, ft * 128 + 128)
                psq = pp.tile([128, SHARD], F32, tag="pp")
                for kc in range(NFT):
                    nc.tensor.matmul(
                        psq[:], Wq_sb[kc][:, csl], xT_sb[kc][:, PAD:PAD + SHARD],
                        start=(kc == 0), stop=(kc == NFT - 1))
                qt = qpool.tile([128, SHARD], DT_ATTN, tag=f"qT{ft}")
                copy_alt(qt[:], psq[:])
                qT_sb.append(qt)

                kt = qpool.tile([128, HALO], DT_ATTN, tag=f"kT{ft}")
                psk = pp.tile([128, SHARD], F32, tag="pp")
                for kc in range(NFT):
                    nc.tensor.matmul(
                        psk[:], Wk_sb[kc][:, csl], xT_sb[kc][:, 0:512],
                        start=(kc == 0), stop=(kc == NFT - 1))
                copy_alt(kt[:, 0:512], psk[:])
                psk2 = pp.tile([128, 64], F32, tag="pp")
                for kc in range(NFT):
                    nc.tensor.matmul(
                        psk2[:], Wk_sb[kc][:, csl], xT_sb[kc][:, 512:HALO],
                        start=(kc == 0), stop=(kc == NFT - 1))
                copy_alt(kt[:, 512:HALO], psk2[:])
                kT_sb.append(kt)

            for tt in range(5):
                rows = 128 if tt < 4 else HALO - 512
                psv = pp.tile([128, DM], F32, tag="pp")
                for kc in range(NFT):
                    nc.tensor.matmul(
                        psv[:rows, :], xT_sb[kc][:, tt * 128:tt * 128 + rows],
                        Wv_sb[kc][:, :],
                        start=(kc == 0), stop=(kc == NFT - 1))
                vt = qpool.tile([128, DM], DT_ATTN, tag=f"v{tt}")
                copy_alt(vt[:rows, :], psv[:rows, :])
                v_sb.append(vt)

            # partition-shifted v: vs[b][64:128] = v_sb[b+1][0:64], so the
            # second 64-key chunk of each strip sits at base partition 64
            v_shift = []
            for b in range(NBLK):
                vs = qpool.tile([128, DM], DT_ATTN, tag=f"vs{b}", name=f"vs{b}")
                nc.sync.dma_start(out=vs[64:128, :], in_=v_sb[b + 1][0:64, :])
                v_shift.append(vs)

            if stage == "proj":
                for tt in range(NBLK):
                    osb = opool.tile([128, DM], F32, tag="osb")
                    nc.vector.tensor_copy(osb[:], v_sb[tt][:])
                    nc.sync.dma_start(
                        out=out_d[tt * 128:(tt + 1) * 128, :], in_=osb[:])

            attnT_sb = [qpool.tile([128, SHARD], DT_ATTN, tag=f"attnT{i}",
                                   name=f"attnT{i}")
                        for i in range(NFT)]

            # ---- attention + per-block output projection ----
            for b in range(NBLK if stage == "full" else 0):
                qsl = slice(b * 128, b * 128 + 128)
                ksl = slice(b * 128, b * 128 + KEYS)
                for hp in range(H // 2):   # head pair: heads 2hp, 2hp+1
                    denom = wpool.tile([128, 2], F32, tag="denom")
                    ems = []
                    for j in range(2):
                        rsl = slice(j * 64, j * 64 + 64)
                        sc = ps.tile([128, KEYS], F32, tag="ps",
                                     name=f"sc{j}")
                        nc.tensor.matmul(
                            sc[:], qT_sb[hp][rsl, qsl], kT_sb[hp][rsl, ksl],
                            start=True, stop=True)
                        e = wpool.tile([128, KEYS], DT_ATTN, tag="e",
                                       name=f"e{j}")
                        nc.scalar.activation(out=e[:], in_=sc[:],
                                             func=Exp, scale=0.125)
                        em = wpool.tile([128, KEYS], DT_ATTN, tag="em",
                                        name=f"em{j}")
                        nc.vector.scalar_tensor_tensor(
                            out=em[:], in0=e[:], scalar=1.0, in1=mask_sb[:],
                            op0=Mult, op1=Mult,
                            accum_out=denom[:, j:j + 1])
                        ems.append(em)
                    recip = wpool.tile([128, 2], F32, tag="recip")
                    nc.vector.reciprocal(recip[:], denom[:])
                    for j in range(2):
                        h = 2 * hp + j
                        hsl = slice(h * 64, (h + 1) * 64)
                        w = wpool.tile([128, KEYS], DT_ATTN, tag="w",
                                       name=f"w{j}")
                        nc.vector.tensor_scalar_mul(
                            w[:], ems[j][:], recip[:, j:j + 1])
                        wT = wpool.tile([128, 256], DT_ATTN, tag="wT",
                                        name=f"wT{j}")
                        nc.sync.dma_start_transpose(wT[:, 0:128], w[:, 0:128])
                        nc.sync.dma_start_transpose(wT[:, 128:256], w[:, 64:KEYS])
                        pav = pa.tile([64, 128], F32, tag="pa")
                        nc.tensor.matmul(
                            pav[:], v_sb[b][:, hsl], wT[:, 0:128],
                            start=True, stop=False)
                        nc.tensor.matmul(
                            pav[:], v_shift[b][64:128, hsl],
                            wT[64:128, 128:256],
                            start=False, stop=True)
                        copy_alt(attnT_sb[hp][j * 64:(j + 1) * 64, qsl],
                                 pav[:])

                # output projection for this block
                po = pp.tile([128, DM], F32, tag="pp")
                for kc in range(NFT):
                    nc.tensor.matmul(
                        po[:], attnT_sb[kc][:, qsl], Wo_sb[kc][:, :],
                        start=(kc == 0), stop=(kc == NFT - 1))
                osb = opool.tile([128, DM], F32, tag="osb")
                nc.vector.tensor_add(osb[:], po[:], bias_sb[:])
                nc.sync.dma_start(out=out_d[b * 128:(b + 1) * 128, :],
                                  in_=osb[:])

    nc.compile()
    return nc


_CACHE = {}


def _get_program():
    if "nc" not in _CACHE:
        _CACHE["nc"] = _build_program()
    return _CACHE["nc"]


def _make_in_maps(x, W_qkv, W_out, b_out):
    np_proj = _np_dt(DT_PROJ)
    np_attn = _np_dt(DT_ATTN)
    Wr = W_qkv.reshape(DM, H, 3, D)
    Wq = np.ascontiguousarray(Wr[:, :, 0, :].reshape(DM, DM), dtype=np_proj)
    Wk = np.ascontiguousarray(Wr[:, :, 1, :].reshape(DM, DM), dtype=np_proj)
    Wv = np.ascontiguousarray(Wr[:, :, 2, :].reshape(DM, DM), dtype=np_proj)
    Wo = np.ascontiguousarray(W_out, dtype=np_attn)
    bias = np.ascontiguousarray(b_out, dtype=np.float32)
    ii = np.arange(128)[:, None]
    kk = np.arange(KEYS)[None, :]
    mask2 = np.ascontiguousarray(
        np.where((kk >= ii) & (kk <= ii + WIN - 1), 1.0, 0.0), dtype=np_attn)

    in_maps = []
    for c in range(NCORES):
        bidx, s0 = c // (NCORES // B), (c % (NCORES // B)) * SHARD
        xh = np.zeros((HALO, DM), np.float32)
        lo, hi = s0 - PAD, s0 + SHARD + PAD
        clo, chi = max(lo, 0), min(hi, S)
        xh[clo - lo:chi - lo] = x[bidx, clo:chi]
        xT = np.ascontiguousarray(xh.T, dtype=np_proj)
        in_maps.append({
            "xT": xT, "Wq": Wq, "Wk": Wk, "Wv": Wv, "Wo": Wo,
            "bias": bias, "mask2": mask2,
        })
    return in_maps


def kernel(x, W_qkv, W_out, b_out, _trace=False, _tmpdir=None):
    x = np.asarray(x, dtype=np.float32)
    W_qkv = np.asarray(W_qkv, dtype=np.float32)
    W_out = np.asarray(W_out, dtype=np.float32)
    b_out = np.asarray(b_out, dtype=np.float32)

    nc = _get_program()
    in_maps = _make_in_maps(x, W_qkv, W_out, b_out)
    res = run_bass_kernel_spmd(
        nc, in_maps, list(range(NCORES)), trace=_trace, tmpdir=_tmpdir)
    _CACHE["last_results"] = res
    out = np.concatenate(
        [res.results[c]["out"] for c in range(NCORES)], axis=0)
    return out.reshape(B, S, DM).astype(np.float32)


# revision 36
# speedup vs baseline: 1.8627x; 1.8627x over previous
"""Local self-attention (window=65) Trainium2 kernel, 8 NeuronCores.

Sharding: 4096 tokens (B*S flattened) split into 8 shards of 512 tokens.
Each core gets a halo'd, pre-transposed x slice plus replicated weights
(halo = 32 tokens each side, zero-padded at batch-sequence edges; zero x
tokens produce exactly-zero k/v since the qkv projection has no bias,
matching the reference's zero-padding semantics).

Per-core pipeline (Bass/Tile, bf16 matmuls with fp32 accumulation):
  1. qT/kT projections (feature-major) and v projection (token-major);
     a partition-shifted copy of v serves the unaligned second half of
     each 192-key strip.
  2. Per 128-query block x head: banded scores [128,192] on PE,
     exp on ACT straight out of PSUM, multiplicative band-mask fused
     with the row-sum in one DVE scalar_tensor_tensor, normalize, then
     transpose the weights via DMA-XBAR (no PE transposes) and run the
     two AV matmuls back into feature-major layout.
  3. Per-block output projection + bias, DMA out.
"""

import numpy as np
import ml_dtypes

import concourse.bass as bass
import concourse.mybir as mybir
import concourse.tile as tile
from concourse import bacc
from concourse.bass_utils import run_bass_kernel_spmd

F32 = mybir.dt.float32
BF16 = mybir.dt.bfloat16

# ---- problem constants (hardcoded) ----
B, S, DM = 2, 2048, 512
H, D, WIN = 8, 64, 65
PAD = WIN // 2              # 32
NCORES = 8
SHARD = B * S // NCORES     # 512 tokens per core
HALO = SHARD + 2 * PAD      # 576
NBLK = SHARD // 128         # 4 query blocks
KEYS = 128 + 2 * PAD        # 192 keys per block
NFT = DM // 128             # 4 feature tiles

DT_PROJ = BF16   # x / Wq / Wk / Wv sbuf dtype (projection matmuls)
DT_ATTN = BF16   # qT/kT/v/e/w/attnT/Wo sbuf dtype (attention + out-proj)


def _np_dt(dt):
    return {F32: np.float32, BF16: ml_dtypes.bfloat16}[dt]


def _build_program(stage="full"):
    nc = bacc.Bacc("TRN2", target_bir_lowering=False, debug=False)

    xT_d = nc.dram_tensor("xT", [DM, HALO], DT_PROJ, kind="ExternalInput")
    Wq_d = nc.dram_tensor("Wq", [DM, DM], DT_PROJ, kind="ExternalInput")
    Wk_d = nc.dram_tensor("Wk", [DM, DM], DT_PROJ, kind="ExternalInput")
    Wv_d = nc.dram_tensor("Wv", [DM, DM], DT_PROJ, kind="ExternalInput")
    Wo_d = nc.dram_tensor("Wo", [DM, DM], DT_ATTN, kind="ExternalInput")
    bias_d = nc.dram_tensor("bias", [DM], F32, kind="ExternalInput")
    mask_d = nc.dram_tensor("mask2", [128, KEYS], DT_ATTN, kind="ExternalInput")
    ident_d = nc.dram_tensor("ident", [128, 128], DT_ATTN, kind="ExternalInput")
    out_d = nc.dram_tensor("out", [SHARD, DM], F32, kind="ExternalOutput")

    Exp = mybir.ActivationFunctionType.Exp
    Copy = mybir.ActivationFunctionType.Copy
    Mult = mybir.AluOpType.mult

    cp_idx = [0]

    def copy_alt(out, in_):
        # alternate PSUM->SBUF copies between DVE and ACT to balance load
        if cp_idx[0] % 2 == 0:
            nc.vector.tensor_copy(out=out, in_=in_)
        else:
            nc.scalar.activation(out=out, in_=in_, func=Copy)
        cp_idx[0] += 1

    with tile.TileContext(nc) as tc:
        with (
            tc.tile_pool(name="consts", bufs=1) as cpool,
            tc.tile_pool(name="qkv", bufs=1) as qpool,
            tc.tile_pool(name="work", bufs=4) as wpool,
            tc.tile_pool(name="outp", bufs=2) as opool,
            tc.tile_pool(name="pp", bufs=2, space="PSUM") as pp,
            tc.tile_pool(name="ps", bufs=2, space="PSUM") as ps,
            tc.tile_pool(name="pw", bufs=2, space="PSUM") as pw,
            tc.tile_pool(name="pa", bufs=2, space="PSUM") as pa,
        ):
            # ---- load constants ----
            xT_sb, Wq_sb, Wk_sb, Wv_sb, Wo_sb = [], [], [], [], []
            for kc in range(NFT):
                t = cpool.tile([128, HALO], DT_PROJ, tag=f"xT{kc}")
                nc.sync.dma_start(out=t[:], in_=xT_d[128 * kc:128 * (kc + 1), :])
                xT_sb.append(t)
            for nm, dram, lst, dt in (
                ("Wq", Wq_d, Wq_sb, DT_PROJ),
                ("Wk", Wk_d, Wk_sb, DT_PROJ),
                ("Wv", Wv_d, Wv_sb, DT_PROJ),
                ("Wo", Wo_d, Wo_sb, DT_ATTN),
            ):
                for kc in range(NFT):
                    t = cpool.tile([128, DM], dt, tag=f"{nm}{kc}")
                    nc.sync.dma_start(out=t[:], in_=dram[128 * kc:128 * (kc + 1), :])
                    lst.append(t)
            bias_sb = cpool.tile([128, DM], F32, tag="bias")
            bias_ap = bias_d[:]
            nc.gpsimd.dma_start(
                out=bias_sb[:],
                in_=bass.AP(tensor=bias_ap.tensor, offset=bias_ap.offset,
                            ap=[[0, 128]] + list(bias_ap.ap)),
            )
            mask_sb = cpool.tile([128, KEYS], DT_ATTN, tag="mask")
            nc.sync.dma_start(out=mask_sb[:], in_=mask_d[:, :])
            ident_sb = cpool.tile([128, 128], DT_ATTN, tag="ident")
            nc.sync.dma_start(out=ident_sb[:], in_=ident_d[:, :])

            # ---- projections ----
            qT_sb, kT_sb, v_sb = [], [], []
            for ft in range(NFT):
                csl = slice(ft * 128, ft * 128 + 128)
                psq = pp.tile([128, SHARD], F32, tag="pp")
                for kc in range(NFT):
                    nc.tensor.matmul(
                        psq[:], Wq_sb[kc][:, csl], xT_sb[kc][:, PAD:PAD + SHARD],
                        start=(kc == 0), stop=(kc == NFT - 1))
                qt = qpool.tile([128, SHARD], DT_ATTN, tag=f"qT{ft}")
                copy_alt(qt[:], psq[:])
                qT_sb.append(qt)

                kt = qpool.tile([128, HALO], DT_ATTN, tag=f"kT{ft}")
                psk = pp.tile([128, SHARD], F32, tag="pp")
                for kc in range(NFT):
                    nc.tensor.matmul(
                        psk[:], Wk_sb[kc][:, csl], xT_sb[kc][:, 0:512],
                        start=(kc == 0), stop=(kc == NFT - 1))
                copy_alt(kt[:, 0:512], psk[:])
                psk2 = pp.tile([128, 64], F32, tag="pp")
                for kc in range(NFT):
                    nc.tensor.matmul(
                        psk2[:], Wk_sb[kc][:, csl], xT_sb[kc][:, 512:HALO],
                        start=(kc == 0), stop=(kc == NFT - 1))
                copy_alt(kt[:, 512:HALO], psk2[:])
                kT_sb.append(kt)

            v_sb = [None] * 5

            def emit_v(tt):
                # v projection for token tile tt (interleaved into the
                # attention stream to keep PE dense), plus the partition-
                # shifted copy vs[tt-1][64:128] = v[tt][0:64] used by the
                # unaligned second 64-key chunk of strip tt-1
                rows = 128 if tt < 4 else HALO - 512
                psv = pp.tile([128, DM], F32, tag="pp", name="psv")
                for kc in range(NFT):
                    nc.tensor.matmul(
                        psv[:rows, :], xT_sb[kc][:, tt * 128:tt * 128 + rows],
                        Wv_sb[kc][:, :],
                        start=(kc == 0), stop=(kc == NFT - 1))
                vt = qpool.tile([128, DM], DT_ATTN, tag=f"v{tt}",
                                name=f"v{tt}")
                copy_alt(vt[:rows, :], psv[:rows, :])
                v_sb[tt] = vt

            emit_v(0)
            emit_v(1)

            if stage == "proj":
                for tt in range(2, 5):
                    emit_v(tt)
                for tt in range(NBLK):
                    osb = opool.tile([128, DM], F32, tag="osb")
                    nc.vector.tensor_copy(osb[:], v_sb[tt][:])
                    nc.sync.dma_start(
                        out=out_d[tt * 128:(tt + 1) * 128, :], in_=osb[:])

            attnT_sb = [qpool.tile([128, SHARD], DT_ATTN, tag=f"attnT{i}",
                                   name=f"attnT{i}")
                        for i in range(NFT)]

            # ---- attention + per-block output projection ----
            for b in range(NBLK if stage == "full" else 0):
                if b + 2 < 5:
                    emit_v(b + 2)   # keeps big matmuls in the PE stream
                qsl = slice(b * 128, b * 128 + 128)
                ksl = slice(b * 128, b * 128 + KEYS)
                for hp in range(H // 2):   # head pair: heads 2hp, 2hp+1
                    denom = wpool.tile([128, 2], F32, tag="denom")
                    ems = []
                    for j in range(2):
                        rsl = slice(j * 64, j * 64 + 64)
                        sc = ps.tile([128, KEYS], F32, tag="ps",
                                     name=f"sc{j}")
                        nc.tensor.matmul(
                            sc[:], qT_sb[hp][rsl, qsl], kT_sb[hp][rsl, ksl],
                            start=True, stop=True)
                        e = wpool.tile([128, KEYS], DT_ATTN, tag="e",
                                       name=f"e{j}")
                        nc.scalar.activation(out=e[:], in_=sc[:],
                                             func=Exp, scale=0.125)
                        em = wpool.tile([128, KEYS], DT_ATTN, tag="em",
                                        name=f"em{j}")
                        nc.vector.scalar_tensor_tensor(
                            out=em[:], in0=e[:], scalar=1.0, in1=mask_sb[:],
                            op0=Mult, op1=Mult,
                            accum_out=denom[:, j:j + 1])
                        ems.append(em)
                    recip = wpool.tile([128, 2], F32, tag="recip")
                    nc.vector.reciprocal(recip[:], denom[:])
                    pav = pa.tile([128, 128], F32, tag="pa")
                    for j in range(2):
                        h = 2 * hp + j
                        hsl = slice(h * 64, (h + 1) * 64)
                        w = wpool.tile([128, KEYS], DT_ATTN, tag="w",
                                       name=f"w{j}")
                        nc.vector.tensor_scalar_mul(
                            w[:], ems[j][:], recip[:, j:j + 1])
                        pwt = pw.tile([128, 256], DT_ATTN, tag="pw",
                                      name=f"pwt{j}")
                        nc.tensor.transpose(
                            pwt[:, 0:128], w[:, 0:128], ident_sb[:])
                        nc.tensor.transpose(
                            pwt[0:64, 128:256], w[:, 128:KEYS], ident_sb[:])
                        wT = wpool.tile([128, 256], DT_ATTN, tag="wT",
                                        name=f"wT{j}")
                        copy_alt(wT[:, 0:128], pwt[:, 0:128])
                        copy_alt(wT[0:64, 128:256], pwt[0:64, 128:256])
                        nc.tensor.matmul(
                            pav[64 * j:64 * (j + 1), :],
                            v_sb[b][:, hsl], wT[:, 0:128],
                            start=True, stop=False)
                        nc.tensor.matmul(
                            pav[64 * j:64 * (j + 1), :],
                            v_sb[b + 1][0:64, hsl],
                            wT[0:64, 128:256],
                            start=False, stop=True)
                    copy_alt(attnT_sb[hp][:, qsl], pav[:])

                # output projection for this block
                po = pp.tile([128, DM], F32, tag="pp")
                for kc in range(NFT):
                    nc.tensor.matmul(
                        po[:], attnT_sb[kc][:, qsl], Wo_sb[kc][:, :],
                        start=(kc == 0), stop=(kc == NFT - 1))
                osb = opool.tile([128, DM], F32, tag="osb")
                nc.vector.tensor_add(osb[:], po[:], bias_sb[:])
                nc.sync.dma_start(out=out_d[b * 128:(b + 1) * 128, :],
                                  in_=osb[:])

    nc.compile()
    return nc


_CACHE = {}


def _get_program():
    if "nc" not in _CACHE:
        _CACHE["nc"] = _build_program()
    return _CACHE["nc"]


def _make_in_maps(x, W_qkv, W_out, b_out):
    np_proj = _np_dt(DT_PROJ)
    np_attn = _np_dt(DT_ATTN)
    Wr = W_qkv.reshape(DM, H, 3, D)
    Wq = np.ascontiguousarray(Wr[:, :, 0, :].reshape(DM, DM), dtype=np_proj)
    Wk = np.ascontiguousarray(Wr[:, :, 1, :].reshape(DM, DM), dtype=np_proj)
    Wv = np.ascontiguousarray(Wr[:, :, 2, :].reshape(DM, DM), dtype=np_proj)
    Wo = np.ascontiguousarray(W_out, dtype=np_attn)
    bias = np.ascontiguousarray(b_out, dtype=np.float32)
    ii = np.arange(128)[:, None]
    kk = np.arange(KEYS)[None, :]
    mask2 = np.ascontiguousarray(
        np.where((kk >= ii) & (kk <= ii + WIN - 1), 1.0, 0.0), dtype=np_attn)
    ident = np.eye(128, dtype=np_attn)

    in_maps = []
    for c in range(NCORES):
        bidx, s0 = c // (NCORES // B), (c % (NCORES // B)) * SHARD
        xh = np.zeros((HALO, DM), np.float32)
        lo, hi = s0 - PAD, s0 + SHARD + PAD
        clo, chi = max(lo, 0), min(hi, S)
        xh[clo - lo:chi - lo] = x[bidx, clo:chi]
        xT = np.ascontiguousarray(xh.T, dtype=np_proj)
        in_maps.append({
            "xT": xT, "Wq": Wq, "Wk": Wk, "Wv": Wv, "Wo": Wo,
            "bias": bias, "mask2": mask2, "ident": ident,
        })
    return in_maps


def kernel(x, W_qkv, W_out, b_out, _trace=False, _tmpdir=None):
    x = np.asarray(x, dtype=np.float32)
    W_qkv = np.asarray(W_qkv, dtype=np.float32)
    W_out = np.asarray(W_out, dtype=np.float32)
    b_out = np.asarray(b_out, dtype=np.float32)

    nc = _get_program()
    in_maps = _make_in_maps(x, W_qkv, W_out, b_out)
    res = run_bass_kernel_spmd(
        nc, in_maps, list(range(NCORES)), trace=_trace, tmpdir=_tmpdir)
    _CACHE["last_results"] = res
    out = np.concatenate(
        [res.results[c]["out"] for c in range(NCORES)], axis=0)
    return out.reshape(B, S, DM).astype(np.float32)


# revision 39
# speedup vs baseline: 1.8787x; 1.0086x over previous
"""Local self-attention (window=65) Trainium2 kernel, 8 NeuronCores.

Sharding: 4096 tokens (B*S flattened) split into 8 shards of 512 tokens.
Each core gets a halo'd, pre-transposed x slice plus replicated weights
(halo = 32 tokens each side, zero-padded at batch-sequence edges; zero x
tokens produce exactly-zero k/v since the qkv projection has no bias,
matching the reference's zero-padding semantics).

Per-core pipeline (Bass/Tile, bf16 matmuls with fp32 accumulation):
  1. qT/kT projections (feature-major) and v projection (token-major);
     a partition-shifted copy of v serves the unaligned second half of
     each 192-key strip.
  2. Per 128-query block x head: banded scores [128,192] on PE,
     exp on ACT straight out of PSUM, multiplicative band-mask fused
     with the row-sum in one DVE scalar_tensor_tensor, normalize, then
     transpose the weights via DMA-XBAR (no PE transposes) and run the
     two AV matmuls back into feature-major layout.
  3. Per-block output projection + bias, DMA out.
"""

import numpy as np
import ml_dtypes

import concourse.bass as bass
import concourse.mybir as mybir
import concourse.tile as tile
from concourse import bacc
from concourse.bass_utils import run_bass_kernel_spmd

F32 = mybir.dt.float32
BF16 = mybir.dt.bfloat16

# ---- problem constants (hardcoded) ----
B, S, DM = 2, 2048, 512
H, D, WIN = 8, 64, 65
PAD = WIN // 2              # 32
NCORES = 8
SHARD = B * S // NCORES     # 512 tokens per core
HALO = SHARD + 2 * PAD      # 576
NBLK = SHARD // 128         # 4 query blocks
KEYS = 128 + 2 * PAD        # 192 keys per block
NFT = DM // 128             # 4 feature tiles

DT_PROJ = BF16   # x / Wq / Wk / Wv sbuf dtype (projection matmuls)
DT_ATTN = BF16   # qT/kT/v/e/w/attnT/Wo sbuf dtype (attention + out-proj)


def _np_dt(dt):
    return {F32: np.float32, BF16: ml_dtypes.bfloat16}[dt]


def _build_program(stage="full"):
    nc = bacc.Bacc("TRN2", target_bir_lowering=False, debug=False)

    xT_d = nc.dram_tensor("xT", [DM, HALO], DT_PROJ, kind="ExternalInput")
    Wq_d = nc.dram_tensor("Wq", [DM, DM], DT_PROJ, kind="ExternalInput")
    Wk_d = nc.dram_tensor("Wk", [DM, DM], DT_PROJ, kind="ExternalInput")
    Wv_d = nc.dram_tensor("Wv", [DM, DM], DT_PROJ, kind="ExternalInput")
    Wo_d = nc.dram_tensor("Wo", [DM, DM], DT_ATTN, kind="ExternalInput")
    bias_d = nc.dram_tensor("bias", [DM], F32, kind="ExternalInput")
    mask_d = nc.dram_tensor("mask2", [128, KEYS], DT_ATTN, kind="ExternalInput")
    ident_d = nc.dram_tensor("ident", [128, 128], DT_ATTN, kind="ExternalInput")
    out_d = nc.dram_tensor("out", [SHARD, DM], F32, kind="ExternalOutput")

    Exp = mybir.ActivationFunctionType.Exp
    Copy = mybir.ActivationFunctionType.Copy
    Mult = mybir.AluOpType.mult

    cp_idx = [0]

    def copy_alt(out, in_):
        # alternate PSUM->SBUF copies between DVE and ACT to balance load
        if cp_idx[0] % 2 == 0:
            nc.vector.tensor_copy(out=out, in_=in_)
        else:
            nc.scalar.activation(out=out, in_=in_, func=Copy)
        cp_idx[0] += 1

    with tile.TileContext(nc) as tc:
        with (
            tc.tile_pool(name="consts", bufs=1) as cpool,
            tc.tile_pool(name="qkv", bufs=1) as qpool,
            tc.tile_pool(name="work", bufs=4) as wpool,
            tc.tile_pool(name="outp", bufs=2) as opool,
            tc.tile_pool(name="pp", bufs=2, space="PSUM") as pp,
            tc.tile_pool(name="ps", bufs=2, space="PSUM") as ps,
            tc.tile_pool(name="pw", bufs=2, space="PSUM") as pw,
            tc.tile_pool(name="pa", bufs=2, space="PSUM") as pa,
        ):
            # ---- load constants ----
            xT_sb, Wq_sb, Wk_sb, Wv_sb, Wo_sb = [], [], [], [], []
            for kc in range(NFT):
                t = cpool.tile([128, HALO], DT_PROJ, tag=f"xT{kc}")
                nc.sync.dma_start(out=t[:], in_=xT_d[128 * kc:128 * (kc + 1), :])
                xT_sb.append(t)
            for nm, dram, lst, dt in (
                ("Wq", Wq_d, Wq_sb, DT_PROJ),
                ("Wk", Wk_d, Wk_sb, DT_PROJ),
                ("Wv", Wv_d, Wv_sb, DT_PROJ),
                ("Wo", Wo_d, Wo_sb, DT_ATTN),
            ):
                for kc in range(NFT):
                    t = cpool.tile([128, DM], dt, tag=f"{nm}{kc}")
                    nc.sync.dma_start(out=t[:], in_=dram[128 * kc:128 * (kc + 1), :])
                    lst.append(t)
            bias_sb = cpool.tile([128, DM], F32, tag="bias")
            bias_ap = bias_d[:]
            nc.gpsimd.dma_start(
                out=bias_sb[:],
                in_=bass.AP(tensor=bias_ap.tensor, offset=bias_ap.offset,
                            ap=[[0, 128]] + list(bias_ap.ap)),
            )
            mask_sb = cpool.tile([128, KEYS], DT_ATTN, tag="mask")
            nc.sync.dma_start(out=mask_sb[:], in_=mask_d[:, :])
            ident_sb = cpool.tile([128, 128], DT_ATTN, tag="ident")
            nc.sync.dma_start(out=ident_sb[:], in_=ident_d[:, :])

            # ---- projections ----
            qT_sb, kT_sb, v_sb = [], [], []
            for ft in range(NFT):
                csl = slice(ft * 128, ft * 128 + 128)
                psq = pp.tile([128, SHARD], F32, tag="pp")
                for kc in range(NFT):
                    nc.tensor.matmul(
                        psq[:], Wq_sb[kc][:, csl], xT_sb[kc][:, PAD:PAD + SHARD],
                        start=(kc == 0), stop=(kc == NFT - 1))
                qt = qpool.tile([128, SHARD], DT_ATTN, tag=f"qT{ft}")
                copy_alt(qt[:], psq[:])
                qT_sb.append(qt)

                kt = qpool.tile([128, HALO], DT_ATTN, tag=f"kT{ft}")
                psk = pp.tile([128, SHARD], F32, tag="pp")
                for kc in range(NFT):
                    nc.tensor.matmul(
                        psk[:], Wk_sb[kc][:, csl], xT_sb[kc][:, 0:512],
                        start=(kc == 0), stop=(kc == NFT - 1))
                copy_alt(kt[:, 0:512], psk[:])
                psk2 = pp.tile([128, 64], F32, tag="pp")
                for kc in range(NFT):
                    nc.tensor.matmul(
                        psk2[:], Wk_sb[kc][:, csl], xT_sb[kc][:, 512:HALO],
                        start=(kc == 0), stop=(kc == NFT - 1))
                copy_alt(kt[:, 512:HALO], psk2[:])
                kT_sb.append(kt)

            v_sb = [None] * 5
            v_dn = [None] * 5

            def emit_v(tt):
                # v projection for token tile tt (interleaved into the
                # attention stream to keep PE dense), plus the partition-
                # shifted copy vs[tt-1][64:128] = v[tt][0:64] used by the
                # unaligned second 64-key chunk of strip tt-1
                rows = 128 if tt < 4 else HALO - 512
                psv = pp.tile([128, DM], F32, tag="pp", name="psv")
                for kc in range(NFT):
                    nc.tensor.matmul(
                        psv[:rows, :], xT_sb[kc][:, tt * 128:tt * 128 + rows],
                        Wv_sb[kc][:, :],
                        start=(kc == 0), stop=(kc == NFT - 1))
                vt = qpool.tile([128, DM], DT_ATTN, tag=f"v{tt}",
                                name=f"v{tt}")
                copy_alt(vt[:rows, :], psv[:rows, :])
                v_sb[tt] = vt
                if tt >= 1:
                    # vdn[t][64:128] = v[t][0:64]: the 64-key tail of each
                    # strip at base partition 64 (pairs with wT rows 64:128)
                    vd = qpool.tile([128, DM], DT_ATTN, tag=f"vdn{tt}",
                                    name=f"vdn{tt}")
                    nc.sync.dma_start(out=vd[64:128, :], in_=vt[0:64, :])
                    v_dn[tt] = vd

            emit_v(0)
            emit_v(1)

            if stage == "proj":
                for tt in range(2, 5):
                    emit_v(tt)
                for tt in range(NBLK):
                    osb = opool.tile([128, DM], F32, tag="osb")
                    nc.vector.tensor_copy(osb[:], v_sb[tt][:])
                    nc.sync.dma_start(
                        out=out_d[tt * 128:(tt + 1) * 128, :], in_=osb[:])

            attnT_sb = [qpool.tile([128, SHARD], DT_ATTN, tag=f"attnT{i}",
                                   name=f"attnT{i}")
                        for i in range(NFT)]

            # ---- attention + per-block output projection ----
            # wpair layout L = [w0 k0:128 | w1 k0:128 | w0 k128:192 | w1
            # k128:192] -> 3 PE transposes per head pair instead of 4:
            #   T1 = L[:,0:128].T   = head0 keys 0:128   (partitions 0:128)
            #   T2 = L[:,128:256].T = head1 keys 0:128   (partitions 0:128)
            #   T3 = L[:,256:384].T = head0 keys 128:192 at partitions 0:64
            #                       + head1 keys 128:192 at partitions 64:128
            pendingB = [None]

            def flushB():
                if pendingB[0] is not None:
                    fn, pendingB[0] = pendingB[0], None
                    fn()

            def stageB(b, hp, L, pav):
                qsl = slice(b * 128, b * 128 + 128)
                pwt = pw.tile([128, 384], DT_ATTN, tag="pw", name="pwt")
                nc.tensor.transpose(pwt[:, 0:128], L[:, 0:128], ident_sb[:])
                nc.tensor.transpose(pwt[:, 128:256], L[:, 128:256],
                                    ident_sb[:])
                nc.tensor.transpose(pwt[:, 256:384], L[:, 256:384],
                                    ident_sb[:])
                wT = wpool.tile([128, 384], DT_ATTN, tag="wT", name="wT")
                copy_alt(wT[:], pwt[:])
                h0 = slice(2 * hp * 64, 2 * hp * 64 + 64)
                h1 = slice((2 * hp + 1) * 64, (2 * hp + 1) * 64 + 64)
                nc.tensor.matmul(pav[0:64, :], v_sb[b][:, h0],
                                 wT[:, 0:128], start=True, stop=False)
                nc.tensor.matmul(pav[0:64, :], v_sb[b + 1][0:64, h0],
                                 wT[0:64, 256:384], start=False, stop=True)
                nc.tensor.matmul(pav[64:128, :], v_sb[b][:, h1],
                                 wT[:, 128:256], start=True, stop=False)
                nc.tensor.matmul(pav[64:128, :], v_dn[b + 1][64:128, h1],
                                 wT[64:128, 256:384], start=False, stop=True)
                copy_alt(attnT_sb[hp][:, qsl], pav[:])

            for b in range(NBLK if stage == "full" else 0):
                if b + 2 < 5:
                    emit_v(b + 2)   # keeps big matmuls in the PE stream
                qsl = slice(b * 128, b * 128 + 128)
                ksl = slice(b * 128, b * 128 + KEYS)
                for hp in range(H // 2):   # head pair: heads 2hp, 2hp+1
                    denom = wpool.tile([128, 2], F32, tag="denom")
                    ems = []
                    for j in range(2):
                        rsl = slice(j * 64, j * 64 + 64)
                        sc = ps.tile([128, KEYS], F32, tag="ps",
                                     name=f"sc{j}")
                        nc.tensor.matmul(
                            sc[:], qT_sb[hp][rsl, qsl], kT_sb[hp][rsl, ksl],
                            start=True, stop=True)
                        e = wpool.tile([128, KEYS], DT_ATTN, tag="e",
                                       name=f"e{j}")
                        nc.scalar.activation(out=e[:], in_=sc[:],
                                             func=Exp, scale=0.125)
                        em = wpool.tile([128, KEYS], DT_ATTN, tag="em",
                                        name=f"em{j}")
                        nc.vector.scalar_tensor_tensor(
                            out=em[:], in0=e[:], scalar=1.0, in1=mask_sb[:],
                            op0=Mult, op1=Mult,
                            accum_out=denom[:, j:j + 1])
                        ems.append(em)
                    recip = wpool.tile([128, 2], F32, tag="recip")
                    nc.vector.reciprocal(recip[:], denom[:])
                    L = wpool.tile([128, 384], DT_ATTN, tag="L", name="L")
                    for j in range(2):
                        nc.vector.tensor_scalar_mul(
                            L[:, 128 * j:128 * (j + 1)],
                            ems[j][:, 0:128], recip[:, j:j + 1])
                        nc.vector.tensor_scalar_mul(
                            L[:, 256 + 64 * j:256 + 64 * (j + 1)],
                            ems[j][:, 128:KEYS], recip[:, j:j + 1])
                    pav = pa.tile([128, 128], F32, tag="pa")
                    flushB()
                    pendingB[0] = (lambda b=b, hp=hp, L=L, pav=pav:
                                   stageB(b, hp, L, pav))

                flushB()
                # output projection for this block
                po = pp.tile([128, DM], F32, tag="pp")
                for kc in range(NFT):
                    nc.tensor.matmul(
                        po[:], attnT_sb[kc][:, qsl], Wo_sb[kc][:, :],
                        start=(kc == 0), stop=(kc == NFT - 1))
                osb = opool.tile([128, DM], F32, tag="osb")
                nc.vector.tensor_add(osb[:], po[:], bias_sb[:])
                nc.sync.dma_start(out=out_d[b * 128:(b + 1) * 128, :],
                                  in_=osb[:])

    nc.compile()
    return nc


_CACHE = {}


def _get_program():
    if "nc" not in _CACHE:
        _CACHE["nc"] = _build_program()
    return _CACHE["nc"]


def _make_in_maps(x, W_qkv, W_out, b_out):
    np_proj = _np_dt(DT_PROJ)
    np_attn = _np_dt(DT_ATTN)
    Wr = W_qkv.reshape(DM, H, 3, D)
    Wq = np.ascontiguousarray(Wr[:, :, 0, :].reshape(DM, DM), dtype=np_proj)
    Wk = np.ascontiguousarray(Wr[:, :, 1, :].reshape(DM, DM), dtype=np_proj)
    Wv = np.ascontiguousarray(Wr[:, :, 2, :].reshape(DM, DM), dtype=np_proj)
    Wo = np.ascontiguousarray(W_out, dtype=np_attn)
    bias = np.ascontiguousarray(b_out, dtype=np.float32)
    ii = np.arange(128)[:, None]
    kk = np.arange(KEYS)[None, :]
    mask2 = np.ascontiguousarray(
        np.where((kk >= ii) & (kk <= ii + WIN - 1), 1.0, 0.0), dtype=np_attn)
    ident = np.eye(128, dtype=np_attn)

    in_maps = []
    for c in range(NCORES):
        bidx, s0 = c // (NCORES // B), (c % (NCORES // B)) * SHARD
        xh = np.zeros((HALO, DM), np.float32)
        lo, hi = s0 - PAD, s0 + SHARD + PAD
        clo, chi = max(lo, 0), min(hi, S)
        xh[clo - lo:chi - lo] = x[bidx, clo:chi]
        xT = np.ascontiguousarray(xh.T, dtype=np_proj)
        in_maps.append({
            "xT": xT, "Wq": Wq, "Wk": Wk, "Wv": Wv, "Wo": Wo,
            "bias": bias, "mask2": mask2, "ident": ident,
        })
    return in_maps


def kernel(x, W_qkv, W_out, b_out, _trace=False, _tmpdir=None):
    x = np.asarray(x, dtype=np.float32)
    W_qkv = np.asarray(W_qkv, dtype=np.float32)
    W_out = np.asarray(W_out, dtype=np.float32)
    b_out = np.asarray(b_out, dtype=np.float32)

    nc = _get_program()
    in_maps = _make_in_maps(x, W_qkv, W_out, b_out)
    res = run_bass_kernel_spmd(
        nc, in_maps, list(range(NCORES)), trace=_trace, tmpdir=_tmpdir)
    _CACHE["last_results"] = res
    out = np.concatenate(
        [res.results[c]["out"] for c in range(NCORES)], axis=0)
    return out.reshape(B, S, DM).astype(np.float32)
